# revision 14
# baseline (speedup 1.0000x reference)
"""NT-Xent contrastive loss on 8 Trainium2 NeuronCores — symmetric version.

sim = z z^T is symmetric, so each unordered block pair is computed once:
core c computes its own 1024 rows against column blocks
[c+1, c+2, c+3, c+4, c] (mod 8, diagonal block last).  Cores 4-7 would
duplicate the distance-4 pairs, so their 4th block is zero padding
(exp(0) = 1 exactly; the host subtracts the constant).  Row sums cover
the computing core's rows; column sums of each exp'd off-diagonal block
cover the partner core's rows.  The host assembles the 8192
denominators from the row/column partials, takes log, and finishes the
loss — the same host-combine role as the baseline, with vectors
instead of scalars.

Per-core device work drops to 5/8 of the full-row scheme on both the
PE and ACT.  Inputs are host-normalized, x16-scaled fp8 e4m3; main
matmuls run in DoubleRow perf mode (K=256 per instruction, one output
column per cycle = 2x bf16).  exp writes fp8 pairs of row tiles so one
DoubleRow ones-matmul per pair produces the column sums; row sums ride
on DVE scalar_tensor_tensor pair-accumulation (off-diag groups) and the
ACT accumulator (diagonal group, so nothing trails the last exp).
"""

import functools
import math

import ml_dtypes
import numpy as np

import concourse.bacc as bacc
import concourse.bass as bass
import concourse.tile as tile
from concourse import mybir
from concourse.bass_utils import run_bass_kernel_spmd
from concourse.hw_specs import get_activation_tables as _orig_gat

F32 = mybir.dt.float32
BF16 = mybir.dt.bfloat16
FP8 = mybir.dt.float8e4
AF = mybir.ActivationFunctionType
ALU = mybir.AluOpType

N_CORES = 8
N = 4096              # rows per input
D = 512               # embedding dim
M = 2 * N             # 8192 rows of sim
ROWS_PER_CORE = M // N_CORES      # 1024
POS_PER_CORE = N // N_CORES       # 512
D_CH = D // 128       # 4 contraction chunks of 128
E2 = float(math.exp(2.0))
INV_T = 2.0           # 1 / temperature
S8 = 16.0             # fp8 pre-scale; psum = S8^2 * cos
EXP_SCALE = INV_T / (S8 * S8)     # 2/256
CW = 1024             # column-block width (2 PSUM banks)
NB = 5                # column blocks per core (diag + 3 + dist4/pad)
CT = NB * CW          # 5120 columns per core
NI = ROWS_PER_CORE // 128         # 8 row tiles

_ONE_SET = "natural_log_exp_and_others"


@functools.cache
def _patched_gat(arch):
    t = dict(_orig_gat(arch))
    if _ONE_SET not in t:
        return t
    mine = {AF.Exp, AF.Ln, AF.Square, AF.Copy, AF.Identity}
    return {
        name: (s if name == _ONE_SET else (set(s) - mine))
        for name, s in t.items()
    }


def build_program():
    bacc.get_activation_tables = _patched_gat

    nc = bacc.Bacc(
        "TRN2",
        target_bir_lowering=False,
        debug=False,
        num_devices=N_CORES,
    )

    zc8 = nc.dram_tensor("zc8", [128, D_CH, CT], FP8, kind="ExternalInput")
    my8 = nc.dram_tensor("my8", [128, D_CH, ROWS_PER_CORE], FP8,
                         kind="ExternalInput")
    pi = nc.dram_tensor("pi", [POS_PER_CORE, D], F32, kind="ExternalInput")
    pj = nc.dram_tensor("pj", [POS_PER_CORE, D], F32, kind="ExternalInput")
    dn_d = nc.dram_tensor("dn8", [128, NI], F32, kind="ExternalOutput")
    cs_d = nc.dram_tensor("cs", [NB, CW], F32, kind="ExternalOutput")
    pos_d = nc.dram_tensor("pos", [128, 1], F32, kind="ExternalOutput")

    with tile.TileContext(nc) as tc:
        import contextlib

        with contextlib.ExitStack() as ctx:
            const = ctx.enter_context(tc.tile_pool(name="const", bufs=1))
            big = ctx.enter_context(tc.tile_pool(name="big", bufs=1))
            esp = ctx.enter_context(tc.tile_pool(name="esp", bufs=12))
            posp = ctx.enter_context(tc.tile_pool(name="posp", bufs=8))
            psnk = ctx.enter_context(tc.tile_pool(name="psnk", bufs=2))
            pp = ctx.enter_context(
                tc.tile_pool(name="pp", bufs=3, space="PSUM")
            )
            csp = ctx.enter_context(
                tc.tile_pool(name="csp", bufs=1, space="PSUM")
            )

            ones_f = const.tile([128, 1], F32)
            nc.vector.memset(ones_f[:], 1.0)
            ones_cs = const.tile([128, 2, 16], FP8)
            nc.vector.memset(ones_cs[:], 1.0)

            zt = big.tile([128, D_CH, CT], FP8, tag="zt")
            myt = big.tile([128, D_CH, ROWS_PER_CORE], FP8, tag="myt")
            dacc = big.tile([128, 2 * NI], F32, tag="dacc")
            pos_dot = big.tile([128, 4], F32, tag="pos_dot")
            cs_sb = [big.tile([1, CW], F32, tag=f"cs_sb{k}",
                              name=f"cs_sb{k}") for k in range(NB)]

            # HAM warmup while the input DMAs stream
            wma = const.tile([128, 128], BF16)
            nc.vector.memset(wma[:], 0.0)
            wmb = const.tile([128, 512], BF16)
            nc.vector.memset(wmb[:], 0.0)
            wmp = pp.tile([128, CW], F32, tag="pp", name="wmp")
            for _ in range(16):
                nc.tensor.matmul(wmp[:, 0:512], wma[:], wmb[:],
                                 start=True, stop=True)

            # --- input DMAs: first column block first -------------------
            nc.sync.dma_start(zt[:, :, 0 : CW // 2], zc8[:, :, 0 : CW // 2])
            nc.sync.dma_start(myt[:], my8[:])
            nc.sync.dma_start(zt[:, :, CW // 2 : CW],
                              zc8[:, :, CW // 2 : CW])
            for g in range(1, NB):
                nc.sync.dma_start(zt[:, :, g * CW : (g + 1) * CW],
                                  zc8[:, :, g * CW : (g + 1) * CW])
            pos_in = []
            for t in range(4):
                pit = posp.tile([128, D], F32, tag="posp")
                nc.sync.dma_start(pit[:], pi[bass.ts(t, 128), :])
                pjt = posp.tile([128, D], F32, tag="posp")
                nc.sync.dma_start(pjt[:], pj[bass.ts(t, 128), :])
                pos_in.append((pit, pjt))

            # --- main loop: 5 column blocks x 8 row tiles ---------------
            def emit_mains(g, i):
                pt = pp.tile([128, CW], F32, tag="pp", name=f"pt_{g}_{i}")
                for t in range(2):
                    lw = myt[:, 2 * t : 2 * t + 2, bass.ts(i, 128)]
                    for jj in range(2):
                        nc.tensor.matmul(
                            pt[:, bass.ts(jj, 512)],
                            lw,
                            zt[:, 2 * t : 2 * t + 2,
                               g * CW + jj * 512 : g * CW + (jj + 1) * 512],
                            start=(t == 0), stop=(t == 1),
                            perf_mode=mybir.MatmulPerfMode.DoubleRow,
                        )
                return pt

            def emit_pos():
                for t in range(4):
                    pit, pjt = pos_in[t]
                    snk = psnk.tile([128, D], F32, tag="psnk")
                    nc.vector.scalar_tensor_tensor(
                        snk[:], pit[:], 1.0, pjt[:],
                        op0=ALU.mult, op1=ALU.mult,
                        accum_out=pos_dot[:, t : t + 1],
                    )
                posr = big.tile([128, 1], F32, tag="posr")
                nc.vector.tensor_reduce(
                    posr[:], pos_dot[:], axis=mybir.AxisListType.X,
                    op=ALU.add
                )
                nc.sync.dma_start(pos_d[:], posr[:])

            held = {}
            cs_ps = {}
            es_cur = {}
            pending = []
            for g in range(NB):
                for i in range(NI):
                    pt = emit_mains(g, i)
                    if pending:
                        pending.pop(0)()
                    if i % 2 == 0:
                        es_cur[g] = esp.tile([128, 2, CW], FP8, tag="es",
                                             name=f"es_{g}_{i}")
                    es2 = es_cur[g]
                    sl = i % 2
                    # diag row sums come from its column sums (the diag
                    # block is symmetric), so every exp is plain
                    nc.scalar.activation(es2[:, sl, :], pt[:], AF.Exp,
                                         scale=EXP_SCALE)
                    if i % 2 == 1:
                        # column sums of the off-diagonal block: one fp8
                        # DoubleRow matmul covers both row tiles of the
                        # pair; emitted a unit behind so the PE FIFO
                        # never head-blocks on exp
                        ip = i // 2
                        if ip == 0:
                            cs_ps[g] = csp.tile([1, CW], F32, tag="csp",
                                                name=f"cs_{g}")
                        def make_cs(g=g, ip=ip, es2=es2):
                            def emit():
                                for jj in range(2):
                                    nc.tensor.matmul(
                                        cs_ps[g][0:1, bass.ts(jj, 512)],
                                        ones_cs[:, :, 0:1],
                                        es2[:, :, bass.ts(jj, 512)],
                                        start=(ip == 0), stop=(ip == 3),
                                        skip_group_check=True,
                                        perf_mode=
                                        mybir.MatmulPerfMode.DoubleRow,
                                    )
                                if ip == 3:
                                    nc.vector.tensor_copy(
                                        cs_sb[g][:], cs_ps[g][0:1, :]
                                    )
                                    nc.sync.dma_start(
                                        cs_d[g : g + 1, :], cs_sb[g][:]
                                    )
                            return emit
                        pending.append(make_cs())
                    # row-sum accumulation: pairs (g0,g1), (g2,g3), g4 solo
                    if g in (0, 2):
                        held[i] = (es2, sl)
                    elif g in (1, 3):
                        h = g // 2
                        k = h * NI + i
                        hes, hsl = held[i]
                        nc.vector.scalar_tensor_tensor(
                            hes[:, hsl, :], hes[:, hsl, :], 1.0,
                            es2[:, sl, :],
                            op0=ALU.mult, op1=ALU.add,
                            accum_out=dacc[:, k : k + 1],
                        )
                    # (diag row sums handled by the exp's accum_out)
                if g == 0:
                    emit_pos()
                if g == 3:
                    # both pair-group accумs done: finalize dn during the
                    # diag phase, off the tail
                    dn = big.tile([128, NI], F32, tag="dn")
                    nc.vector.tensor_reduce(
                        dn[:], dacc[:].rearrange("p (h i) -> p i h", h=2),
                        axis=mybir.AxisListType.X, op=ALU.add,
                    )
                    nc.sync.dma_start(dn_d[:], dn[:])
            for p in pending:
                p()

    nc.compile()
    return nc


_NC_CACHE = None


def _get_program():
    global _NC_CACHE
    if _NC_CACHE is None:
        _NC_CACHE = build_program()
    return _NC_CACHE


def _block_list(c):
    bl = [(c + 1) % 8, (c + 2) % 8, (c + 3) % 8]
    bl.append((c + 4) % 8 if c < 4 else -1)     # -1 = zero pad
    bl.append(c)                                 # diag last (no col sums)
    return bl


def make_in_maps(emb_i: np.ndarray, emb_j: np.ndarray):
    emb_i = np.asarray(emb_i, dtype=np.float32)
    emb_j = np.asarray(emb_j, dtype=np.float32)
    reps = np.concatenate([emb_i, emb_j], axis=0).astype(np.float64)
    z = reps / np.sqrt((reps * reps).sum(axis=1, keepdims=True))
    zT = np.ascontiguousarray(z.T * S8)                    # [512, 8192]
    z8_full = np.ascontiguousarray(
        zT.reshape(D_CH, 128, M).transpose(1, 0, 2)
    ).astype(ml_dtypes.float8_e4m3)                        # [128, 4, 8192]
    pad = np.zeros((128, D_CH, CW), dtype=ml_dtypes.float8_e4m3)
    zf = z.astype(np.float32)
    in_maps = []
    for c in range(N_CORES):
        parts = []
        for b in _block_list(c):
            if b < 0:
                parts.append(pad)
            else:
                parts.append(z8_full[:, :, b * CW : (b + 1) * CW])
        zc8 = np.ascontiguousarray(np.concatenate(parts, axis=2))
        in_maps.append(
            {
                "zc8": zc8,
                "my8": np.ascontiguousarray(
                    z8_full[:, :, c * CW : (c + 1) * CW]
                ),
                "pi": np.ascontiguousarray(
                    zf[c * POS_PER_CORE : (c + 1) * POS_PER_CORE]
                ),
                "pj": np.ascontiguousarray(
                    zf[N + c * POS_PER_CORE : N + (c + 1) * POS_PER_CORE]
                ),
            }
        )
    return in_maps


def combine_outputs(results):
    total = np.zeros(M, dtype=np.float64)
    cos_sum = 0.0
    for c, r in enumerate(results):
        dn8 = np.asarray(r["dn8"], dtype=np.float64)       # [128, 8]
        rows = dn8.T.reshape(-1)                           # row = i*128+p
        total[c * CW : (c + 1) * CW] += rows
        if c >= 4:
            total[c * CW : (c + 1) * CW] -= float(CW)      # pad exp(0)=1
        cs = np.asarray(r["cs"], dtype=np.float64)         # [5, 1024]
        for k in range(NB):
            b = _block_list(c)[k]
            if b < 0:
                continue
            total[b * CW : (b + 1) * CW] += cs[k]
        cos_sum += float(np.asarray(r["pos"], dtype=np.float64).sum())
    denom = total - E2
    loss = (np.log(denom).sum() - 2.0 * INV_T * cos_sum) / float(M)
    return np.float32(loss)


def kernel(emb_i: np.ndarray, emb_j: np.ndarray) -> np.ndarray:
    nc = _get_program()
    in_maps = make_in_maps(emb_i, emb_j)
    res = run_bass_kernel_spmd(nc, in_maps, list(range(N_CORES)))
    return combine_outputs(res.results)
